# revision 5
# baseline (speedup 1.0000x reference)
"""Trainium2 Bass kernel for StyleGAN2-style modulated conv (groups=batch).

Per-core program (data-parallel over batch, 8 cores, 1 sample each):
  1. style: s[ci] = c_lin * (w_b @ lin_weight[ci, :]) + lin_bias[ci]
     folded:  t[ci] = c_conv * s[ci]
  2. modulated weights (transposed for matmul): WT[ci, k, co] = conv_weight[co, ci, k] * t[ci]
  3. demod: sumsq[co] = sum_ci t[ci]^2 * Q[co, ci],  Q = sum_k conv_weight^2
     sigma_inv[co] = 1/sqrt(sumsq + EPS)   (applied to conv OUTPUT - linearity)
  4. conv 3x3 pad 1 as 36 shifted matmuls per (co_tile, hw_chunk) accumulated in PSUM,
     output scaled by sigma_inv[co] on PSUM->SBUF eviction.
"""

import math
import os

import numpy as np

import concourse.bass as bass
import concourse.tile as tile
from concourse import mybir
from concourse.bass_utils import run_bass_kernel_spmd
from concourse.masks import make_identity

B, C_IN, C_OUT, H, W, KS, W_DIM = 8, 512, 512, 64, 64, 3, 512
EPS = 1e-8
PAD = 1
HP, WP = H + 2 * PAD, W + 2 * PAD  # 66, 66
N_CORES = 8
CI_T = C_IN // 128   # 4 ci tiles
CO_T = C_OUT // 128  # 4 co tiles
K2 = KS * KS         # 9
ROWS_PER_CHUNK = 8   # output rows per psum tile -> N = 8*64 = 512
N_CHUNKS = H // ROWS_PER_CHUNK  # 8

F32 = mybir.dt.float32
F32R = mybir.dt.float32r
BF16 = mybir.dt.bfloat16

# matmul dtype config: "f32" (exact, 4 cyc/row), "f32r" (1 cyc/row @ N>=256),
# "bf16" (1 cyc/row, operands rounded to bf16)
MM_DTYPE = os.environ.get("TRNK_DTYPE", "f32r")


def _split_multi_waits(nc):
    """This walrus build accepts at most one semaphore wait per instruction.

    Tile freely attaches several; split the extras onto preceding NoOps on
    the same engine (program order preserved, so semantics are identical).
    """
    n = 0
    for f in nc.m.functions:
        for blk in f.blocks:
            insts = list(blk.instructions)
            new, changed = [], False
            for inst in insts:
                si = inst.sync_info
                if si is not None and si.on_wait is not None and len(si.on_wait) > 1:
                    waits = list(si.on_wait)
                    for k, w in enumerate(waits[:-1]):
                        new.append(
                            mybir.InstNoOp(
                                name=f"{inst.name}_wsplit{k}",
                                engine=inst.engine,
                                sync_info=mybir.SyncInfo(on_wait=[w], on_update=[]),
                                bass_nofuse=True,
                            )
                        )
                        n += 1
                    inst.sync_info = mybir.SyncInfo(
                        on_wait=[waits[-1]], on_update=list(si.on_update or [])
                    )
                    changed = True
                new.append(inst)
            if changed:
                blk.instructions = new
    return n


def _build(mm_dtype: str):
    c_lin = 1.0 / math.sqrt(W_DIM)
    c_conv = 1.0 / math.sqrt(C_IN * KS * KS)

    store_dt = BF16 if mm_dtype == "bf16" else F32
    mm_cast = F32R if mm_dtype == "f32r" else None

    nc = bass.Bass()
    xb = nc.dram_tensor("xb", [C_IN, H, W], F32, kind="ExternalInput")
    wb = nc.dram_tensor("wb", [1, W_DIM], F32, kind="ExternalInput")
    cw = nc.dram_tensor("cw", [C_OUT, C_IN * K2], F32, kind="ExternalInput")
    lw = nc.dram_tensor("lw", [C_IN, W_DIM], F32, kind="ExternalInput")
    lb = nc.dram_tensor("lb", [C_IN], F32, kind="ExternalInput")
    y = nc.dram_tensor("y", [C_OUT, H, W], F32, kind="ExternalOutput")

    with tile.TileContext(nc) as tc:
        with (
            tc.tile_pool(name="const", bufs=1) as constp,
            tc.tile_pool(name="small", bufs=1) as smallp,
            tc.tile_pool(name="xp", bufs=1) as xpp,
            tc.tile_pool(name="xstage", bufs=1) as xsp,
            tc.tile_pool(name="cw", bufs=2) as cwp,
            tc.tile_pool(name="sq", bufs=1) as sqp,
            tc.tile_pool(name="scr", bufs=2) as scrp,
            tc.tile_pool(name="q", bufs=1) as qp,
            tc.tile_pool(name="wt", bufs=1) as wtp,
            tc.tile_pool(name="yst", bufs=3) as yp,
            tc.tile_pool(name="ps_conv", bufs=3, space="PSUM") as ps_conv,
            tc.tile_pool(name="ps_tr", bufs=2, space="PSUM") as ps_tr,
            tc.tile_pool(name="ps_small", bufs=1, space="PSUM") as ps_small,
        ):
            # ---- constants ----
            ident = constp.tile([128, 128], F32, tag="ident")
            make_identity(nc, ident[:])
            ones1 = constp.tile([1, 128], F32, tag="ones1")
            nc.vector.memset(ones1[:], 1.0)

            # ---- style path: t_col[ci] = c_conv*(c_lin*(w_b . lw[ci,:]) + bias[ci]) ----
            wbb = smallp.tile([128, W_DIM], F32, tag="wbb")
            nc.sync.dma_start(wbb[:], wb[0:1, :].partition_broadcast(128))
            bias_col = smallp.tile([128, CI_T], F32, tag="bias_col")
            for i in range(CI_T):
                nc.sync.dma_start(
                    bias_col[:, i : i + 1], lb[i * 128 : (i + 1) * 128].unsqueeze(1)
                )
            t_col = smallp.tile([128, CI_T], F32, tag="t_col")
            s_col = smallp.tile([128, CI_T], F32, tag="s_col")
            sumsq_col = smallp.tile([128, CO_T], F32, tag="sumsq_col")
            sig_tmp = smallp.tile([128, CO_T], F32, tag="sig_tmp")
            sigma_col = smallp.tile([128, CO_T], F32, tag="sigma_col")
            t_row = smallp.tile([1, C_IN], F32, tag="t_row")
            t2_row = smallp.tile([1, C_IN], F32, tag="t2_row")
            t2b = smallp.tile([128, C_IN], F32, tag="t2b")
            bias_sc = smallp.tile([128, CI_T], F32, tag="bias_sc")
            nc.scalar.mul(bias_sc[:], bias_col[:], c_conv)
            eps_col = smallp.tile([128, 1], F32, tag="eps_col")
            nc.vector.memset(eps_col[:], float(EPS))

            for i in range(CI_T):
                lw_t = smallp.tile([128, W_DIM], F32, tag="lw")
                nc.sync.dma_start(lw_t[:], lw[i * 128 : (i + 1) * 128, :])
                scr = scrp.tile([128, W_DIM], F32, tag="sscr")
                nc.vector.tensor_mul(scr[:], lw_t[:], wbb[:])
                nc.vector.reduce_sum(
                    s_col[:, i : i + 1], scr[:], axis=mybir.AxisListType.X
                )
                # t = c_lin*c_conv*s_raw + c_conv*bias
                nc.scalar.activation(
                    t_col[:, i : i + 1],
                    s_col[:, i : i + 1],
                    mybir.ActivationFunctionType.Identity,
                    bias=bias_sc[:, i : i + 1],
                    scale=c_lin * c_conv,
                )
                # t_row via PE transpose [128,1] -> [1,128]
                pr = ps_small.tile([1, 128], F32, tag="pr")
                nc.tensor.transpose(pr[:], t_col[:, i : i + 1], ident[:])
                nc.scalar.copy(t_row[0:1, i * 128 : (i + 1) * 128], pr[:])

            nc.vector.tensor_mul(t2_row[:], t_row[:], t_row[:])
            p2 = ps_small.tile([128, C_IN], F32, tag="p2")
            nc.tensor.matmul(p2[:], ones1[:], t2_row[:], start=True, stop=True)
            nc.scalar.copy(t2b[:], p2[:])

            # ---- x load + zero-pad (to 66x66) ----
            xp_tiles = []
            for i in range(CI_T):
                xpt = xpp.tile([128, HP * WP], store_dt, tag=f"xp{i}")
                xp_tiles.append(xpt)
                nc.vector.memset(xpt[:], 0.0)
                xs = xsp.tile([128, H * W], F32, tag="xs")
                nc.sync.dma_start(xs[:], xb[i * 128 : (i + 1) * 128, :, :])
                xp_v = xpt[:].rearrange("p (r c) -> p r c", c=WP)
                xs_v = xs[:].rearrange("p (r c) -> p r c", c=W)
                nc.vector.tensor_copy(
                    xp_v[:, PAD : PAD + H, PAD : PAD + W], xs_v[:, :, :]
                )

            # ---- per co_tile: weight prep + conv ----
            for ct in range(CO_T):
                co0 = ct * 128
                cw_t = cwp.tile([128, C_IN * K2], F32, tag="cw")
                nc.sync.dma_start(cw_t[:], cw[co0 : co0 + 128, :])
                cw_v = cw_t[:].rearrange("p (ci k) -> p ci k", k=K2)

                # demod: Q = sum_k cw^2 ; sumsq = sum_ci t2*Q (+EPS); sigma=1/sqrt
                sq_t = sqp.tile([128, C_IN * K2], F32, tag="sq")
                nc.vector.tensor_mul(sq_t[:], cw_t[:], cw_t[:])
                q_t = qp.tile([128, C_IN], F32, tag="q")
                sq_v = sq_t[:].rearrange("p (ci k) -> p ci k", k=K2)
                nc.vector.reduce_sum(q_t[:], sq_v[:, :, :], axis=mybir.AxisListType.X)
                qscr = scrp.tile([128, C_IN], F32, tag="sscr")
                nc.vector.tensor_mul(qscr[:], q_t[:], t2b[:])
                nc.vector.reduce_sum(
                    sumsq_col[:, ct : ct + 1], qscr[:], axis=mybir.AxisListType.X
                )
                # sigma = 1/sqrt(sumsq + EPS)
                nc.scalar.activation(
                    sig_tmp[:, ct : ct + 1],
                    sumsq_col[:, ct : ct + 1],
                    mybir.ActivationFunctionType.Sqrt,
                    bias=eps_col[:, :],
                )
                nc.vector.reciprocal(
                    sigma_col[:, ct : ct + 1], sig_tmp[:, ct : ct + 1]
                )

                # transpose weights into WT[ci_tile][128ci, k*128+co] with t[ci] scaling
                wt_tiles = []
                for ci in range(CI_T):
                    wt_t = wtp.tile([128, K2 * 128], store_dt, tag=f"wt{ci}")
                    wt_tiles.append(wt_t)
                    for k in range(K2):
                        ptr = ps_tr.tile([128, 128], F32, tag="ptr")
                        nc.tensor.transpose(
                            ptr[:], cw_v[:, ci * 128 : (ci + 1) * 128, k], ident[:]
                        )
                        nc.scalar.activation(
                            wt_t[:, k * 128 : (k + 1) * 128],
                            ptr[:],
                            mybir.ActivationFunctionType.Copy,
                            scale=t_col[:, ci : ci + 1],
                        )

                # conv: accumulate 36 matmuls per 8-row chunk
                for ch in range(N_CHUNKS):
                    h0 = ch * ROWS_PER_CHUNK
                    pc = ps_conv.tile([128, ROWS_PER_CHUNK * W], F32, tag="pc")
                    first = True
                    for ci in range(CI_T):
                        xp_v = xp_tiles[ci][:].rearrange("p (r c) -> p r c", c=WP)
                        for k in range(K2):
                            kh, kw = divmod(k, KS)
                            rhs = xp_v[
                                :, h0 + kh : h0 + kh + ROWS_PER_CHUNK, kw : kw + W
                            ]
                            lhsT = wt_tiles[ci][:, k * 128 : (k + 1) * 128]
                            if mm_cast is not None:
                                rhs = rhs.bitcast(mm_cast)
                                lhsT = lhsT.bitcast(mm_cast)
                            nc.tensor.matmul(
                                pc[:],
                                lhsT,
                                rhs,
                                start=first,
                                stop=(ci == CI_T - 1 and k == K2 - 1),
                            )
                            first = False
                    y_t = yp.tile([128, ROWS_PER_CHUNK * W], F32, tag="y")
                    nc.scalar.activation(
                        y_t[:],
                        pc[:],
                        mybir.ActivationFunctionType.Copy,
                        scale=sigma_col[:, ct : ct + 1],
                    )
                    nc.sync.dma_start(
                        y[co0 : co0 + 128, h0 : h0 + ROWS_PER_CHUNK, :], y_t[:]
                    )

    _split_multi_waits(nc)
    return nc


_NC_CACHE = {}


def _get_nc():
    if MM_DTYPE not in _NC_CACHE:
        _NC_CACHE[MM_DTYPE] = _build(MM_DTYPE)
    return _NC_CACHE[MM_DTYPE]


LAST_RESULTS = None


def kernel(x, w, conv_weight, lin_weight, lin_bias):
    global LAST_RESULTS
    nc = _get_nc()
    x = np.ascontiguousarray(np.asarray(x, dtype=np.float32))
    w = np.ascontiguousarray(np.asarray(w, dtype=np.float32))
    cw = np.ascontiguousarray(
        np.asarray(conv_weight, dtype=np.float32).reshape(C_OUT, C_IN * K2)
    )
    lw = np.ascontiguousarray(np.asarray(lin_weight, dtype=np.float32))
    lb = np.ascontiguousarray(np.asarray(lin_bias, dtype=np.float32))

    in_maps = []
    for b in range(N_CORES):
        in_maps.append(
            {
                "xb": x[b],
                "wb": w[b : b + 1],
                "cw": cw,
                "lw": lw,
                "lb": lb,
            }
        )
    trace = os.environ.get("TRNK_TRACE", "0") == "1"
    LAST_RESULTS = run_bass_kernel_spmd(
        nc, in_maps, core_ids=list(range(N_CORES)), trace=trace
    )
    out = np.stack([LAST_RESULTS.results[b]["y"] for b in range(N_CORES)])
    return out.astype(np.float32)
